# revision 49
# baseline (speedup 1.0000x reference)
"""Trainium2 Bass kernel for nn_CausalSelfAttention_61795989455492.

Sharding (8 cores): core c -> batch b = c//2, head-group hg = c%2 (8 of 16
heads). Each core runs QKV projection (its head slice), rotary, sliding-window
attention with joint prefix softmax, and a partial output projection over its
512 channel columns. Host sums the two partials per batch (pair reduce).

Device layout ("transposed attention"):
  - q^T, k^T: (d on partition, t on free) straight out of projection matmuls;
    d-rows are PERMUTED per head ([0:16,32:48,16:32,48:64]) so the rotary
    half-swap becomes a within-32-partition DVE stream_shuffle.
  - att^T blocks: (s on partition, t on free); psum slots bank-packed as
    bank0=[slot0(384)|slot3(128)] bank1=[slot1(384)] bank2=[slot2(256)] so
    diag/bound mask matmuls batch two-at-a-time within a bank.
  - softmax denominator via ones-column appended to V (y_aug row 65);
    reciprocal = exp(-ln(denom)) on ScalarE; broadcast across the 64 d-rows
    via a partition-broadcast SBUF->SBUF DMA (no PE involvement).
  - exp via ScalarE with scale=1/sqrt(D), bf16 out; no max subtraction
  - bf16 operands everywhere (PE runs f32r at bf16 speed; halves DMA/DVE)
  - attention processes TWO heads (one kts pair) interleaved to keep the PE
    dense enough that the HAM clock-gate holds 2.4 GHz.
  - input DMA dispatches split across the two HWDGE queues (sync + scalar).
"""

import sys
from contextlib import ExitStack

import numpy as np

sys.path.insert(0, "/opt/trn_rl_repo")

import ml_dtypes  # noqa: E402
import concourse.bass as bass  # noqa: E402
import concourse.tile as tile_mod  # noqa: E402
from concourse import bacc  # noqa: E402
from concourse import mybir  # noqa: E402

B, T, C, H, D = 4, 512, 1024, 16, 64
S_PREV, PFX, WINDOW = 1536, 256, 256
ROPE_BASE = 10000.0
MASKVAL = -1.0e5
HPC = 8  # heads per core
NCORES = 8

f32 = mybir.dt.float32
f32r = mybir.dt.float32r
bf16 = mybir.dt.bfloat16

# window geometry per 512-col KV chunk, transposed layout:
# s-block tj -> t-run [T0[tj], T0[tj]+TN[tj]) at psum offset POFF[tj]
# (bank-packed: no matmul write crosses a 2KB PSUM bank; diag masks sit at
# +0 and bound masks at +256 of their slot so pairs batch within a bank)
# exp packs slots contiguously into ex at offsets EOFF
_T0 = [0, 128, 256, 384]
_TN = [384, 384, 256, 128]
_POFF = [0, 512, 1024, 384]
_EOFF = [0, 512, 896, 384]
_APSW = 1280
_EXPW = 1152

# d-permutation within each head (so rotary swap is within 32 partitions)
_PERM64 = np.concatenate([np.arange(0, 16), np.arange(32, 48),
                          np.arange(16, 32), np.arange(48, 64)])
_SHUF_MASK = list(range(16, 32)) + list(range(0, 16))

# constpack column layout (bf16): cos2 | sin2 | ident | diag | combo | ones4
# combo holds diag at +0 and bound at +256 (gap keeps the [128,2,128] mask AP
# non-contiguous so it isn't flattened and matches the strided psum out)
_CP_COS, _CP_SIN, _CP_ID, _CP_DG, _CP_CB, _CP_O4 = 0, 512, 1024, 1152, 1280, 1792
_CPW = 1796

USE_DMA_BCAST = True  # partition-broadcast SBUF->SBUF DMA for 1/denom


def _emit(nc, tc, io):
    ctx = ExitStack()
    with ctx:
        const = ctx.enter_context(tc.tile_pool(name="const", bufs=1))
        qkrot = ctx.enter_context(tc.tile_pool(name="qkrot", bufs=1))
        vsb = ctx.enter_context(tc.tile_pool(name="vsb", bufs=1))
        ysb = ctx.enter_context(tc.tile_pool(name="ysb", bufs=1))
        vau_p = ctx.enter_context(tc.tile_pool(name="vau", bufs=1))
        pref_p = ctx.enter_context(tc.tile_pool(name="pref", bufs=1))

        # ---- input DMA: phase-1 critical tensors first, split in c-halves
        # so the first projection matmuls start as soon as ~1MB has landed ----
        def half_pair(nm):
            a = const.tile([128, 4, 512], bf16, name=nm + "a", tag=nm + "a")
            b = const.tile([128, 4, 512], bf16, name=nm + "b", tag=nm + "b")
            return a, b

        x_h = half_pair("xt")
        wq_h = half_pair("wq")
        wk_h = half_pair("wk")
        kts = [const.tile([128, 1536], bf16, name=f"kts{i}", tag=f"kts{i}")
               for i in range(4)]
        nc.sync.dma_start(out=x_h[0], in_=io["xT"].ap()[:, 0:4, :])
        nc.sync.dma_start(out=wq_h[0], in_=io["wq"].ap()[:, 0:4, :])
        nc.sync.dma_start(out=x_h[1], in_=io["xT"].ap()[:, 4:8, :])
        nc.sync.dma_start(out=wq_h[1], in_=io["wq"].ap()[:, 4:8, :])
        nc.sync.dma_start(out=wk_h[0], in_=io["wk"].ap()[:, 0:4, :])
        nc.sync.dma_start(out=kts[0], in_=io["kT_cache"].ap()[:, 0, :])
        nc.sync.dma_start(out=wk_h[1], in_=io["wk"].ap()[:, 4:8, :])
        wv_t = const.tile([128, 8, 512], bf16, name="wv", tag="wv")
        nc.sync.dma_start(out=wv_t, in_=io["wv"].ap())
        cp = const.tile([128, _CPW], bf16, name="constpack", tag="constpack")
        nc.sync.dma_start(out=cp, in_=io["constpack"].ap())
        nc.sync.dma_start(out=kts[1], in_=io["kT_cache"].ap()[:, 1, :])

        def xs(c):
            return x_h[c // 4][:, c % 4, :]

        w_h = [wq_h, wk_h]

        # ---- attention-side tensors, same queue so x/wq get bandwidth first ----
        vau = []
        pref = []
        for h in range(HPC):
            va = vau_p.tile([128, 18, 65], bf16, name=f"vau{h}", tag=f"vau{h}")
            vau.append(va)
            pf = pref_p.tile([128, 1024], bf16, name=f"pref{h}", tag=f"pref{h}")
            pref.append(pf)
        for h in range(HPC):
            nc.sync.dma_start(out=vau[h][:, 0:14, :], in_=io["vpack"].ap()[h])
            nc.sync.dma_start(out=pref[h], in_=io["prefT"].ap()[h])
            if h == 3:
                nc.sync.dma_start(out=kts[2], in_=io["kT_cache"].ap()[:, 2, :])
            if h == 5:
                nc.sync.dma_start(out=kts[3], in_=io["kT_cache"].ap()[:, 3, :])

        sb_cos = cp[:, _CP_COS:_CP_COS + 512]
        sb_sin = cp[:, _CP_SIN:_CP_SIN + 512]
        sb_I = cp[:, _CP_ID:_CP_ID + 128]
        sb_diag = cp[:, _CP_DG:_CP_DG + 128]
        sb_bound = cp[:, _CP_CB + 256:_CP_CB + 384]
        ones4 = cp[:, _CP_O4:_CP_O4 + 4].rearrange("p (a b) -> p a b", b=1)

        q_rot = [qkrot.tile([128, 512], bf16, name=f"qrot{i}", tag=f"qrot{i}") for i in range(4)]
        k_rot = [qkrot.tile([128, 512], bf16, name=f"krot{i}", tag=f"krot{i}") for i in range(4)]
        v_sb = [vsb.tile([128, 512], bf16, name=f"vsb{i}", tag=f"vsb{i}") for i in range(4)]
        y_t = [ysb.tile([128, 512], bf16, name=f"ysb{i}", tag=f"ysb{i}") for i in range(4)]
        _DBG.update(q_rot=q_rot, k_rot=k_rot, v_sb=v_sb, y_t=y_t, vau=vau)

        # ---------------- phase 1: qkv projection + rotary ----------------
        with tc.tile_pool(name="tmp", bufs=2) as tmp, \
             tc.tile_pool(name="projps", bufs=4, space="PSUM") as projps:
            # (section, mtile); v-items are ('v', tb)
            order = [(0, 0), (1, 0), (0, 1), (1, 1), (2, 0), (0, 2), (1, 2),
                     (2, 1), (0, 3), (1, 3), (2, 2), (2, 3)]
            for sect, m in order:
                ps = projps.tile([128, 512], f32, name="projps", tag="projps")
                if sect < 2:
                    for c in range(8):
                        nc.tensor.matmul(
                            ps,
                            lhsT=w_h[sect][c // 4][:, c % 4, m * 128:(m + 1) * 128],
                            rhs=xs(c),
                            start=(c == 0),
                            stop=(c == 7),
                        )
                    # rotary: rot = qk * cos2 + shuffle(qk) * sin2
                    rot = q_rot[m] if sect == 0 else k_rot[m]
                    qsb = tmp.tile([128, 512], bf16, name="qsb", tag="qsb")
                    nc.vector.tensor_copy(qsb, ps)
                    sh = tmp.tile([128, 512], bf16, name="sh", tag="sh")
                    nc.vector.stream_shuffle(sh, qsb, _SHUF_MASK)
                    nc.vector.tensor_mul(rot, qsb, sb_cos)
                    nc.vector.tensor_mul(sh, sh, sb_sin)
                    nc.vector.tensor_add(rot, rot, sh)
                else:
                    tb = m
                    for c in range(8):
                        nc.tensor.matmul(
                            ps,
                            lhsT=xs(c)[:, tb * 128:(tb + 1) * 128],
                            rhs=wv_t[:, c, :],
                            start=(c == 0),
                            stop=(c == 7),
                        )
                    nc.vector.tensor_copy(v_sb[tb], ps)

        wp = const.tile([128, 4, 1024], bf16, name="wp", tag="wp")
        nc.sync.dma_start(out=wp, in_=io["w_projT"].ap())

        # ---------------- phase 2: attention, two heads interleaved ----------------
        with tc.tile_pool(name="expsb", bufs=4) as exp_p, \
             tc.tile_pool(name="exppref", bufs=2) as expp_p, \
             tc.tile_pool(name="rcp", bufs=4) as rcp_p, \
             tc.tile_pool(name="attps", bufs=2, space="PSUM") as attps_p, \
             tc.tile_pool(name="yaug", bufs=2, space="PSUM") as yaug_p:
            pending = []

            def qk_masks(h, ck):
                hrow = (h % 2) * 64
                mt = h // 2
                aps = attps_p.tile([128, _APSW], f32, name="attps", tag="attps")

                def qk(tj):
                    t0, tn, off = _T0[tj], _TN[tj], _POFF[tj]
                    if ck < 3:
                        kblk = kts[mt][hrow:hrow + 64,
                                       ck * 512 + tj * 128: ck * 512 + (tj + 1) * 128]
                    else:
                        kblk = k_rot[mt][hrow:hrow + 64, tj * 128:(tj + 1) * 128]
                    nc.tensor.matmul(
                        aps[:, off:off + tn],
                        lhsT=kblk,
                        rhs=q_rot[mt][hrow:hrow + 64, t0:t0 + tn],
                        start=True, stop=False, skip_group_check=True,
                    )

                # bank0: slot0 then its masks, then slot3 (whose start=True
                # re-arms bank0's pending-zero) then its diag
                qk(0)
                nc.tensor.matmul(
                    aps[:, 0:128], lhsT=sb_I, rhs=sb_diag,
                    start=False, stop=False, skip_group_check=True,
                )
                nc.tensor.matmul(
                    aps[:, 256:384], lhsT=sb_I, rhs=sb_bound,
                    start=False, stop=False, skip_group_check=True,
                )
                qk(3)
                nc.tensor.matmul(
                    aps[:, 384:512], lhsT=sb_I, rhs=sb_diag,
                    start=False, stop=False, skip_group_check=True,
                )
                qk(1)
                nc.tensor.matmul(
                    aps[:, 512:640], lhsT=sb_I, rhs=sb_diag,
                    start=False, stop=False, skip_group_check=True,
                )
                nc.tensor.matmul(
                    aps[:, 768:896], lhsT=sb_I, rhs=sb_bound,
                    start=False, stop=False, skip_group_check=True,
                )
                qk(2)
                nc.tensor.matmul(
                    aps[:, 1024:1152], lhsT=sb_I, rhs=sb_diag,
                    start=False, stop=True, skip_group_check=True,
                )
                # exp (scale=1/sqrt(D)), bf16 out, slots packed into ex
                ex = exp_p.tile([128, _EXPW], bf16, name="expsb", tag="expsb")
                nc.scalar.activation(
                    out=ex[:, 0:896], in_=aps[:, 0:896],
                    func=mybir.ActivationFunctionType.Exp, scale=0.125,
                )
                nc.scalar.activation(
                    out=ex[:, 896:1152], in_=aps[:, 1024:1280],
                    func=mybir.ActivationFunctionType.Exp, scale=0.125,
                )
                return ex

            def av(yps, h, ex, ck, last):
                for tj in range(4):
                    t0, tn, eoff = _T0[tj], _TN[tj], _EOFF[tj]
                    blk = (2 + ck * 4 + tj) if ck < 3 else (14 + tj)
                    nc.tensor.matmul(
                        yps[0:65, t0:t0 + tn],
                        lhsT=vau[h][:, blk, :],
                        rhs=ex[:, eoff:eoff + tn],
                        start=False,
                        stop=(last and tj == 3),
                        skip_group_check=True,
                    )

            def flush_pending():
                while pending:
                    rb_o, yunn_o, mt_o, hrow_o = pending.pop(0)
                    nc.vector.tensor_mul(
                        y_t[mt_o][hrow_o:hrow_o + 64, :], yunn_o, rb_o)

            for p in range(4):
                A, B = 2 * p, 2 * p + 1
                exs = {}
                yp = {}
                for h in (A, B):
                    for tb in range(4):
                        nc.vector.tensor_copy(
                            vau[h][:, 14 + tb, 0:64], v_sb[tb][:, h * 64:(h + 1) * 64])
                    nc.vector.tensor_copy(vau[h][:, 14:18, 64:65], ones4)
                    expp = expp_p.tile([128, 1024], bf16, name="exppref", tag="exppref")
                    nc.scalar.activation(out=expp, in_=pref[h],
                                         func=mybir.ActivationFunctionType.Exp)
                    exs[h, "pfx"] = expp

                # round 0: both heads' QK, then prefix AV (independent of the
                # chunk exps) fills the PE while exp(·,0) completes — no
                # pair-start bubble to drop the HAM clock gate
                for h in (A, B):
                    exs[h, 0] = qk_masks(h, 0)
                for h in (A, B):
                    yps = yaug_p.tile([128, 512], f32, name="yaug", tag="yaug")
                    yp[h] = yps
                    for pb in range(2):
                        nc.tensor.matmul(
                            yps[0:65, :],
                            lhsT=vau[h][:, pb, :],
                            rhs=exs[h, "pfx"][:, pb * 512:(pb + 1) * 512],
                            start=(pb == 0), stop=False,
                            skip_group_check=True,
                        )
                for ck in range(1, 4):
                    for h in (A, B):
                        exs[h, ck] = qk_masks(h, ck)
                        av(yp[h], h, exs[h, ck - 1], ck - 1, False)
                        del exs[h, ck - 1]
                    if ck == 1:
                        flush_pending()  # previous pair's normalize (DVE)
                for h in (A, B):
                    av(yp[h], h, exs[h, 3], 3, True)
                    # 1/denom = exp(-ln(denom)) on ScalarE; broadcast via DMA
                    hrow = (h % 2) * 64
                    mt = h // 2
                    den = rcp_p.tile([1, 512], f32, name="den", tag="den")
                    nc.vector.tensor_copy(den, yp[h][64:65, :])
                    rcp1 = rcp_p.tile([1, 512], f32, name="rcp", tag="rcp")
                    nc.vector.reciprocal_approx_fast(out=rcp1, in_=den)
                    yunn = rcp_p.tile([64, 512], bf16, name="yunn", tag="yunn")
                    nc.vector.tensor_copy(yunn, yp[h][0:64, :])
                    rb = rcp_p.tile([64, 512], f32, name="rb", tag="rb")
                    nc.gpsimd.partition_broadcast(out_ap=rb[:, :], in_ap=rcp1[:, :])
                    pending.append((rb, yunn, mt, hrow))
            flush_pending()

        # ---------------- phase 3: output projection (partial) ----------------
        with tc.tile_pool(name="outsb", bufs=3) as out_p, \
             tc.tile_pool(name="cpps", bufs=3, space="PSUM") as cpps_p:
            for tb in range(4):
                for ng in range(2):
                    cps = cpps_p.tile([128, 512], f32, name="cpps", tag="cpps")
                    for ct in range(4):
                        nc.tensor.matmul(
                            cps,
                            lhsT=y_t[ct][:, tb * 128:(tb + 1) * 128],
                            rhs=wp[:, ct, ng * 512:(ng + 1) * 512],
                            start=(ct == 0),
                            stop=(ct == 3),
                        )
                    ob = out_p.tile([128, 512], f32, name="outsb", tag="outsb")
                    nc.scalar.copy(ob, cps)
                    nc.sync.dma_start(
                        out=io["out"].ap()[tb * 128:(tb + 1) * 128, ng * 512:(ng + 1) * 512],
                        in_=ob,
                    )


def build_nc():
    nc = bacc.Bacc("TRN2", target_bir_lowering=False, debug=False)
    io = {}
    io["xT"] = nc.declare_dram_parameter("xT", [128, 8, 512], bf16, isOutput=False)
    for nm in ("wq", "wk", "wv"):
        io[nm] = nc.declare_dram_parameter(nm, [128, 8, 512], bf16, isOutput=False)
    io["constpack"] = nc.declare_dram_parameter("constpack", [128, _CPW], bf16, isOutput=False)
    io["kT_cache"] = nc.declare_dram_parameter("kT_cache", [128, 4, 1536], bf16, isOutput=False)
    io["vpack"] = nc.declare_dram_parameter("vpack", [HPC, 128, 14, 65], bf16, isOutput=False)
    io["prefT"] = nc.declare_dram_parameter("prefT", [HPC, 128, 1024], bf16, isOutput=False)
    io["w_projT"] = nc.declare_dram_parameter("w_projT", [128, 4, 1024], bf16, isOutput=False)
    io["out"] = nc.declare_dram_parameter("out", [512, 1024], f32, isOutput=True)

    with tile_mod.TileContext(nc) as tc:
        _emit(nc, tc, io)
    nc.finalize()
    return nc


def _rotary_tables(start_index):
    half = D // 2
    inv_freq = 1.0 / (ROPE_BASE ** (np.arange(half, dtype=np.float32) / half))
    pos = (float(start_index) + np.arange(T, dtype=np.float32))
    ang = inv_freq[:, None] * pos[None, :]  # (32, 512): [d, t]
    c = np.cos(ang, dtype=np.float32)
    s = np.sin(ang, dtype=np.float32)
    cos2 = np.tile(c, (4, 1))  # (128, 512)
    sin2 = np.tile(np.concatenate([-s, s], axis=0), (2, 1))  # (128, 512)
    perm128 = np.concatenate([_PERM64, 64 + _PERM64])
    return cos2[perm128], sin2[perm128]


def _constpack(start_index):
    cos2, sin2 = _rotary_tables(start_index)
    i = np.arange(128)
    ident = np.eye(128, dtype=np.float32)
    diag = np.where(i[:, None] > i[None, :], MASKVAL, 0.0)
    bound = np.where(i[None, :] > i[:, None], MASKVAL, 0.0)
    cpk = np.empty((128, _CPW), dtype=ml_dtypes.bfloat16)
    cpk[:, _CP_COS:_CP_COS + 512] = cos2
    cpk[:, _CP_SIN:_CP_SIN + 512] = sin2
    cpk[:, _CP_ID:_CP_ID + 128] = ident
    cpk[:, _CP_DG:_CP_DG + 128] = diag
    cpk[:, _CP_CB:_CP_CB + 512] = 0.0
    cpk[:, _CP_CB:_CP_CB + 128] = diag
    cpk[:, _CP_CB + 256:_CP_CB + 384] = bound
    cpk[:, _CP_O4:_CP_O4 + 4] = 1.0
    return np.ascontiguousarray(cpk)


def make_in_maps(x, c_attn_w, c_proj_w, cached_k, cached_v, att_prefix, cache_v, start_index):
    cpk = _constpack(np.asarray(start_index).item())
    qk_perm = np.concatenate([64 * h + _PERM64 for h in range(HPC)])
    bfc = ml_dtypes.bfloat16

    def tile8(mat):  # (1024, 512) -> (128, 8, 512)
        return np.ascontiguousarray(
            mat.reshape(8, 128, 512).transpose(1, 0, 2)).astype(bfc)

    in_maps = []
    for core in range(NCORES):
        b, hg = core // 2, core % 2
        hs = slice(hg * HPC, (hg + 1) * HPC)
        r0, r1 = hg * 512, (hg + 1) * 512
        wq = c_attn_w[r0:r1][qk_perm]
        wk = c_attn_w[C + r0:C + r1][qk_perm]
        wv = c_attn_w[2 * C + r0:2 * C + r1]
        p = att_prefix[b, hs].transpose(0, 2, 1)  # (8, 256, 512)
        prefT = np.ascontiguousarray(
            np.concatenate([p[:, :128], p[:, 128:]], axis=2)).astype(bfc)
        kb = cached_k[b, hs][:, :, _PERM64]  # (8, 1536, 64) perm d
        kT = kb.transpose(0, 2, 1).reshape(4, 128, 1536)  # head-pairs
        kT_cache = np.ascontiguousarray(kT.transpose(1, 0, 2)).astype(bfc)
        vp = np.concatenate([
            cache_v[b, hs].reshape(HPC, 2, 128, D),
            cached_v[b, hs].reshape(HPC, 12, 128, D),
        ], axis=1)  # (8, 14, 128, 64)
        vpack = np.empty((HPC, 128, 14, 65), dtype=bfc)
        vpack[:, :, :, 0:64] = vp.transpose(0, 2, 1, 3).astype(bfc)
        vpack[:, :, :, 64] = 1.0
        wpm = c_proj_w[:, r0:r1].T  # (512, 1024)
        w_projT = np.ascontiguousarray(
            wpm.reshape(4, 128, 1024).transpose(1, 0, 2)).astype(bfc)
        in_maps.append({
            "xT": tile8(x[b].T),
            "wq": tile8(wq.T),
            "wk": tile8(wk.T),
            "wv": tile8(wv.T),
            "constpack": cpk,
            "kT_cache": kT_cache,
            "vpack": np.ascontiguousarray(vpack),
            "prefT": prefT,
            "w_projT": w_projT,
        })
    return in_maps


_NC_CACHE = {}
_DBG = {}


def kernel(x, c_attn_w, c_proj_w, cached_k, cached_v, att_prefix, cache_v, start_index):
    x = np.asarray(x, dtype=np.float32)
    c_attn_w = np.asarray(c_attn_w, dtype=np.float32)
    c_proj_w = np.asarray(c_proj_w, dtype=np.float32)
    cached_k = np.asarray(cached_k, dtype=np.float32)
    cached_v = np.asarray(cached_v, dtype=np.float32)
    att_prefix = np.asarray(att_prefix, dtype=np.float32)
    cache_v = np.asarray(cache_v, dtype=np.float32)

    if "nc" not in _NC_CACHE:
        _NC_CACHE["nc"] = build_nc()
    nc = _NC_CACHE["nc"]

    in_maps = make_in_maps(x, c_attn_w, c_proj_w, cached_k, cached_v,
                           att_prefix, cache_v, start_index)
    from concourse.bass_utils import run_bass_kernel_spmd
    res = run_bass_kernel_spmd(nc, in_maps, list(range(NCORES)))
    outs = res.results
    y = np.empty((B, T, C), dtype=np.float32)
    for b in range(B):
        y[b] = outs[2 * b]["out"] + outs[2 * b + 1]["out"]
    return y
